# revision 28
# baseline (speedup 1.0000x reference)
"""Trainium2 Bass kernel for nn_CBF (GCN message passing over a radius graph).

8-core SPMD: core c owns agent block m in [c*1024, (c+1)*1024).
Each core builds its [8192 x 1024] column strip of the symmetric adjacency
mask exactly (elementwise fp32, matching jnp rounding), aggregates with the
normalized adjacency via fp8 matmuls on the PE, and all-gathers the small
[N,64] feature matrix between the two GCN layers.

Outputs per core: mask_cols [8192, 1024] u8 (full mask = concat axis=1,
valid because the mask is symmetric... actually because the tiles are
row-indexed by the global k axis directly), out_row [1, 1024] f32.
"""

import numpy as np

import concourse.bass as bass
import concourse.bacc as bacc
import concourse.mybir as mybir
import concourse.tile as tile
from concourse import bass_utils

dt = mybir.dt
Alu = mybir.AluOpType
Act = mybir.ActivationFunctionType

N = 8192
NCORES = 8
MBLK = N // NCORES          # 1024 agents per core
KT = N // 128               # 64 k-tiles of 128 partitions
HID = 64
OBS_R2 = 1.0

_CACHE = {}


def build_nc():
    nc = bacc.Bacc("TRN2", target_bir_lowering=False, debug=False,
                   num_devices=NCORES)

    # ---- kernel I/O (per-core) ----
    xm_in = nc.dram_tensor("xm_in", [128, MBLK], dt.float32, kind="ExternalInput").ap()
    ym_in = nc.dram_tensor("ym_in", [128, MBLK], dt.float32, kind="ExternalInput").ap()
    xk_in = nc.dram_tensor("xk_in", [128, KT], dt.float32, kind="ExternalInput").ap()
    yk_in = nc.dram_tensor("yk_in", [128, KT], dt.float32, kind="ExternalInput").ap()
    stT_in = nc.dram_tensor("stT_in", [4, N], dt.float32, kind="ExternalInput").ap()
    w1_in = nc.dram_tensor("w1_in", [4, HID], dt.float32, kind="ExternalInput").ap()
    w2_in = nc.dram_tensor("w2_in", [HID, HID], dt.bfloat16, kind="ExternalInput").ap()
    b1_in = nc.dram_tensor("b1_in", [HID, 1], dt.float32, kind="ExternalInput").ap()
    b2_in = nc.dram_tensor("b2_in", [HID, 1], dt.float32, kind="ExternalInput").ap()
    cw_in = nc.dram_tensor("cw_in", [HID, 1], dt.float32, kind="ExternalInput").ap()
    cb_in = nc.dram_tensor("cb_in", [1, 1], dt.float32, kind="ExternalInput").ap()
    id64_in = nc.dram_tensor("id64_in", [HID, HID], dt.float32, kind="ExternalInput").ap()

    mask_out = nc.dram_tensor("mask_cols", [N, MBLK], dt.uint8, kind="ExternalOutput").ap()
    out_row = nc.dram_tensor("out_row", [1, MBLK], dt.float32, kind="ExternalOutput").ap()

    with tile.TileContext(nc) as tc:
        with (
            tc.tile_pool(name="const", bufs=1) as cpool,
            tc.tile_pool(name="big", bufs=1) as big,
            tc.tile_pool(name="work", bufs=2) as work,
            tc.tile_pool(name="tmp", bufs=1) as tmp,
            tc.tile_pool(name="dram", bufs=1, space="DRAM") as dram,
        ):
            # ---- load constants (xm/ym split over 4 DMAs to parallelize) --
            xm = cpool.tile([128, MBLK], dt.float32, name="xm")
            ym = cpool.tile([128, MBLK], dt.float32, name="ym")
            for q in range(4):
                sl = slice(q * MBLK // 4, (q + 1) * MBLK // 4)
                nc.sync.dma_start(xm[:, sl], xm_in[:, sl])
                nc.sync.dma_start(ym[:, sl], ym_in[:, sl])
            xk = cpool.tile_from(xk_in)            # [128, 64] f32
            yk = cpool.tile_from(yk_in)
            id64 = cpool.tile_from(id64_in)        # [64, 64] f32 identity
            w1 = cpool.tile_from(w1_in)            # [4, 64] f32
            w2 = cpool.tile_from(w2_in)            # [64, 64] bf16
            b1 = cpool.tile_from(b1_in)            # [64, 1] f32
            b2 = cpool.tile_from(b2_in)
            cw = cpool.tile_from(cw_in)
            cb = cpool.tile_from(cb_in)

            ones8 = cpool.tile([128, 1], dt.float8e4)
            nc.gpsimd.memset(ones8[:], 1.0)
            ones32 = cpool.tile([1, HID], dt.float32)
            nc.gpsimd.memset(ones32[:], 1.0)

            # ---- persistent big tensors ----
            mask8 = big.tile([128, KT * MBLK], dt.float8e4, name="mask8")   # 64 KB/p
            xw1q = big.tile([128, KT * HID], dt.bfloat16, name="xw1q")      # 8 KB/p
            xw2q = big.tile([128, KT * HID], dt.bfloat16, name="xw2q")      # 8 KB/p
            h1full = big.tile([HID, N], dt.bfloat16, name="h1full")         # 16 KB/p
            dinvk = big.tile([128, KT], dt.float32, name="dinvk")
            dinvbc = big.tile([HID, MBLK], dt.float32, name="dinvbc")

            # ================= PHASE 1: mask + deg (+ XW1 on PE) =========
            ph1 = tc.alloc_tile_pool(name="ph1", bufs=1)
            stT = ph1.tile_from(stT_in)                          # [4, 8192] f32
            xw1f = ph1.tile([128, KT * HID], dt.bfloat16, name="xw1f")  # 8 KB/p
            with tc.tile_pool(name="p1ps", bufs=1, space="PSUM") as p1ps:
                degps = p1ps.tile([1, MBLK], dt.float32, name="degps")

                for t in range(KT):
                    sqx = work.tile([128, MBLK], dt.float32, tag="sqx")
                    sqy = work.tile([128, MBLK], dt.float32, tag="sqy")
                    d2 = work.tile([128, MBLK], dt.float32, tag="d2")
                    # (x_k - x_m)^2 : exact fp32 (scale=-1 mult is exact,
                    # then one rounded add, then Square = one rounded mult)
                    if t % 8 in (1, 4, 6):
                        # ACT/DVE load balance: do the x square on DVE
                        dxt = work.tile([128, MBLK], dt.float32, tag="d2")
                        nc.vector.tensor_scalar(dxt[:], xm[:], xk[:, t:t + 1],
                                                None, Alu.subtract)
                        nc.vector.tensor_tensor(sqx[:], dxt[:], dxt[:], Alu.mult)
                    else:
                        nc.scalar.activation(sqx[:], xm[:], Act.Square,
                                             bias=xk[:, t:t + 1], scale=-1.0)
                    nc.scalar.activation(sqy[:], ym[:], Act.Square,
                                         bias=yk[:, t:t + 1], scale=-1.0)
                    # the d2 add: alternate DVE / GpSimd to spread load
                    if t % 2 == 0:
                        nc.gpsimd.tensor_tensor(d2[:], sqx[:], sqy[:], Alu.add)
                    else:
                        nc.vector.tensor_tensor(d2[:], sqx[:], sqy[:], Alu.add)
                    # mask tile in fp8 (0.0 / 1.0); doubles as the bool
                    # output (raw bytes 0x00/0x38, host decodes with != 0)
                    msl = mask8[:, t * MBLK:(t + 1) * MBLK]
                    nc.vector.tensor_scalar(msl, d2[:], OBS_R2, None, Alu.is_le)
                    nc.gpsimd.dma_start(mask_out[t * 128:(t + 1) * 128, :],
                                        msl.bitcast(dt.uint8))
                    # deg += ones^T @ mask_tile  (exact integer sums in fp32)
                    for h in range(2):
                        nc.tensor.matmul(
                            degps[:, h * 512:(h + 1) * 512],
                            ones8[:],
                            msl[:, h * 512:(h + 1) * 512],
                            start=(t == 0), stop=(t == KT - 1))

                # XW1[k, c] = states @ W1 (fp32 PE), packed 8 k-tiles per bank
                with tc.tile_pool(name="xwps", bufs=2, space="PSUM") as xwps:
                    for g in range(KT // 8):
                        ps = xwps.tile([128, 512], dt.float32, tag="xw")
                        for j in range(8):
                            t = g * 8 + j
                            nc.tensor.matmul(
                                ps[:, j * HID:(j + 1) * HID],
                                stT[:, t * 128:(t + 1) * 128],
                                w1[:], start=True, stop=True)
                        nc.scalar.copy(xw1f[:, g * 512:(g + 1) * 512], ps[:])

                # ---- dinv = deg^-1/2 = exp(-0.5 * ln(deg)) (deg is exact) --
                lgdeg = tmp.tile([1, MBLK], dt.float32, tag="sm")
                dinvm = cpool.tile([1, MBLK], dt.float32)
                nc.scalar.activation(lgdeg[:], degps[:], Act.Ln)
                nc.scalar.activation(dinvm[:], lgdeg[:], Act.Exp, scale=-0.5)

            # ---- AllGather dinv across the 8 cores ----
            dinv_src = dram.tile([1, MBLK], dt.float32, name="dinv_src")
            dinv_all = dram.tile([NCORES, MBLK], dt.float32, name="dinv_all",
                                 addr_space="Shared")
            nc.sync.dma_start(dinv_src[:], dinvm[:])
            nc.gpsimd.collective_compute(
                "AllGather", Alu.bypass,
                replica_groups=[list(range(NCORES))],
                ins=[dinv_src[:].opt()], outs=[dinv_all[:].opt()])
            # dinvk[p, t] = dinv[t*128 + p]: contiguous [64,128] load, then
            # PE transpose (a direct strided gather DMA costs ~11us)
            dinvT = tmp.tile([KT, 128], dt.float32, tag="dinvT")
            nc.sync.dma_start(
                dinvT[:],
                dinv_all[:].rearrange("a b -> (a b)").rearrange("(t p) -> t p", p=128))
            with tc.tile_pool(name="trps", bufs=1, space="PSUM") as trps:
                dkps = trps.tile([128, KT], dt.float32, name="dkps")
                nc.tensor.transpose(dkps[:], dinvT[:], id64[:])
                nc.scalar.copy(dinvk[:], dkps[:])

            # dinvbc[c, m] = dinv_m[m] broadcast over 64 partitions (PE trick)
            with tc.tile_pool(name="bcps", bufs=1, space="PSUM") as bcps:
                dbc = bcps.tile([HID, MBLK], dt.float32, name="dbc")
                for h in range(2):
                    nc.tensor.matmul(dbc[:, h * 512:(h + 1) * 512], ones32[:],
                                     dinvm[:, h * 512:(h + 1) * 512],
                                     start=True, stop=True)
                nc.scalar.copy(dinvbc[:], dbc[:])

            # XW1' = dinv[k] * XW1[k, :]  -> fp8
            for t in range(KT):
                nc.vector.tensor_scalar(
                    xw1q[:, t * HID:(t + 1) * HID],
                    xw1f[:, t * HID:(t + 1) * HID],
                    dinvk[:, t:t + 1], None, Alu.mult)
            ph1.release()

            # ================= LAYER 1 SpMM =================
            h1p = cpool.tile([HID, MBLK], dt.bfloat16)
            with tc.tile_pool(name="aggps", bufs=1, space="PSUM") as aggps:
                agg = aggps.tile([HID, MBLK], dt.float32, name="agg1")
                for t in range(KT):
                    for h in range(2):
                        nc.tensor.matmul(
                            agg[:, h * 512:(h + 1) * 512],
                            xw1q[:, t * HID:(t + 1) * HID],
                            mask8[:, t * MBLK + h * 512: t * MBLK + (h + 1) * 512],
                            start=(t == 0), stop=(t == KT - 1))
                # h1 = relu(dinv_m * agg + b1);  h1' = dinv_m * h1 (bf16)
                t1 = tmp.tile([HID, MBLK], dt.float32, tag="tt")
                h1T = tmp.tile([HID, MBLK], dt.float32, tag="hh")
                nc.vector.tensor_tensor(t1[:], agg[:], dinvbc[:], Alu.mult)
                nc.scalar.activation(h1T[:], t1[:], Act.Relu, bias=b1[:])
                nc.vector.tensor_tensor(h1p[:], h1T[:], dinvbc[:], Alu.mult)

            # ---- AllGather h1' ----
            h1_src = dram.tile([HID, MBLK], dt.bfloat16, name="h1_src")
            h1_all = dram.tile([NCORES * HID, MBLK], dt.bfloat16, name="h1_all",
                               addr_space="Shared")
            nc.sync.dma_start(h1_src[:], h1p[:])
            nc.gpsimd.collective_compute(
                "AllGather", Alu.bypass,
                replica_groups=[list(range(NCORES))],
                ins=[h1_src[:].opt()], outs=[h1_all[:].opt()])
            nc.sync.dma_start(
                h1full[:].rearrange("p (r m) -> p r m", r=NCORES),
                h1_all[:].rearrange("(r p) m -> p r m", p=HID))

            # XW2' = h1'^T stationary @ W2 (bf16) -> fp8 (dinv already folded)
            with tc.tile_pool(name="xwps2", bufs=2, space="PSUM") as xwps2:
                for g in range(KT // 8):
                    ps = xwps2.tile([128, 512], dt.float32, tag="xw2")
                    for j in range(8):
                        t = g * 8 + j
                        nc.tensor.matmul(
                            ps[:, j * HID:(j + 1) * HID],
                            h1full[:, t * 128:(t + 1) * 128],
                            w2[:], start=True, stop=True)
                    nc.scalar.copy(xw2q[:, g * 512:(g + 1) * 512], ps[:])

            # ================= LAYER 2 SpMM =================
            h2T = tmp.tile([HID, MBLK], dt.float32, tag="hh")
            with tc.tile_pool(name="aggps2", bufs=1, space="PSUM") as aggps2:
                agg2 = aggps2.tile([HID, MBLK], dt.float32, name="agg2")
                for t in range(KT):
                    for h in range(2):
                        nc.tensor.matmul(
                            agg2[:, h * 512:(h + 1) * 512],
                            xw2q[:, t * HID:(t + 1) * HID],
                            mask8[:, t * MBLK + h * 512: t * MBLK + (h + 1) * 512],
                            start=(t == 0), stop=(t == KT - 1))
                t2 = tmp.tile([HID, MBLK], dt.float32, tag="tt")
                nc.vector.tensor_tensor(t2[:], agg2[:], dinvbc[:], Alu.mult)
                nc.scalar.activation(h2T[:], t2[:], Act.Relu, bias=b2[:])

            # ================= output head =================
            with tc.tile_pool(name="outps", bufs=1, space="PSUM") as outps:
                ops = outps.tile([1, MBLK], dt.float32, name="ops")
                for h in range(2):
                    nc.tensor.matmul(ops[:, h * 512:(h + 1) * 512], cw[:],
                                     h2T[:, h * 512:(h + 1) * 512],
                                     start=True, stop=True)
                osb = tmp.tile([1, MBLK], dt.float32, tag="sm")
                nc.scalar.activation(osb[:], ops[:], Act.Identity, bias=cb[:])
                nc.sync.dma_start(out_row[:], osb[:])

    nc.compile()
    return nc


def _prep_inputs(states, W1, b1, W2, b2, cw, cb):
    states = np.asarray(states, dtype=np.float32)
    x = np.ascontiguousarray(states[:, 0])
    y = np.ascontiguousarray(states[:, 1])
    bf16 = mybir.dt.np(dt.bfloat16)

    xk = np.ascontiguousarray(x.reshape(KT, 128).T)   # [128, 64]
    yk = np.ascontiguousarray(y.reshape(KT, 128).T)
    stT = np.ascontiguousarray(states.T)              # [4, 8192]
    w1 = np.asarray(W1, dtype=np.float32)
    w2 = np.asarray(W2, dtype=np.float32).astype(bf16)
    b1c = np.asarray(b1, dtype=np.float32).reshape(HID, 1)
    b2c = np.asarray(b2, dtype=np.float32).reshape(HID, 1)
    cwc = np.asarray(cw, dtype=np.float32).reshape(HID, 1)
    cbc = np.asarray(cb, dtype=np.float32).reshape(1, 1)
    id64 = np.eye(HID, dtype=np.float32)

    in_maps = []
    for c in range(NCORES):
        xm = np.ascontiguousarray(
            np.broadcast_to(x[c * MBLK:(c + 1) * MBLK][None, :], (128, MBLK)))
        ym = np.ascontiguousarray(
            np.broadcast_to(y[c * MBLK:(c + 1) * MBLK][None, :], (128, MBLK)))
        in_maps.append({
            "xm_in": xm, "ym_in": ym, "xk_in": xk, "yk_in": yk,
            "stT_in": stT, "w1_in": w1, "w2_in": w2,
            "b1_in": b1c, "b2_in": b2c, "cw_in": cwc, "cb_in": cbc,
            "id64_in": id64,
        })
    return in_maps


def _get_nc():
    if "nc" not in _CACHE:
        _CACHE["nc"] = build_nc()
    return _CACHE["nc"]


def _install_ntff_hook():
    """Recreate the antenv.axon_hooks shim this image is missing."""
    import sys, types
    try:
        from antenv.axon_hooks import get_axon_ntff_profile_hook  # noqa
        return
    except ImportError:
        pass
    try:
        import antenv
        sys.path.insert(0, "/root/.axon_site/trn_agent_boot")
        import trn_boot
        hook = trn_boot._ntff_profile_via_ctypes("/opt/axon/libaxon_pjrt.so")
        mod = types.ModuleType("antenv.axon_hooks")
        mod._hook = hook
        mod.get_axon_ntff_profile_hook = lambda: mod._hook
        mod.set_axon_ntff_profile_hook = lambda h: setattr(mod, "_hook", h)
        sys.modules["antenv.axon_hooks"] = mod
        antenv.axon_hooks = mod
    except Exception as e:  # tracing is best-effort
        print("ntff hook install failed:", e)


def run(trace=False, **inputs):
    if trace:
        _install_ntff_hook()
    nc = _get_nc()
    in_maps = _prep_inputs(**inputs)
    res = bass_utils.run_bass_kernel_spmd(
        nc, in_maps, core_ids=list(range(NCORES)), trace=trace)
    mask = np.concatenate([r["mask_cols"] for r in res.results], axis=1)
    out = np.concatenate(
        [r["out_row"].reshape(MBLK, 1) for r in res.results], axis=0)
    return (out.astype(np.float32), mask.astype(bool)), res


def kernel(**inputs):
    (out, mask), _ = run(trace=False, **inputs)
    return out, mask


# revision 29
# speedup vs baseline: 1.2026x; 1.2026x over previous
"""Trainium2 Bass kernel for nn_CBF (GCN message passing over a radius graph).

8-core SPMD: core c owns agent block m in [c*1024, (c+1)*1024).
Each core builds its [8192 x 1024] column strip of the symmetric adjacency
mask exactly (elementwise fp32, matching jnp rounding), aggregates with the
normalized adjacency via fp8 matmuls on the PE, and all-gathers the small
[N,64] feature matrix between the two GCN layers.

Outputs per core: mask_cols [8192, 1024] u8 (full mask = concat axis=1,
valid because the mask is symmetric... actually because the tiles are
row-indexed by the global k axis directly), out_row [1, 1024] f32.
"""

import numpy as np

import concourse.bass as bass
import concourse.bacc as bacc
import concourse.mybir as mybir
import concourse.tile as tile
from concourse import bass_utils

dt = mybir.dt
Alu = mybir.AluOpType
Act = mybir.ActivationFunctionType

N = 8192
NCORES = 8
MBLK = N // NCORES          # 1024 agents per core
KT = N // 128               # 64 k-tiles of 128 partitions
HID = 64
OBS_R2 = 1.0

_CACHE = {}


def build_nc():
    nc = bacc.Bacc("TRN2", target_bir_lowering=False, debug=False,
                   num_devices=NCORES)

    # ---- kernel I/O (per-core) ----
    xm_in = nc.dram_tensor("xm_in", [128, MBLK], dt.float32, kind="ExternalInput").ap()
    ym_in = nc.dram_tensor("ym_in", [128, MBLK], dt.float32, kind="ExternalInput").ap()
    xk_in = nc.dram_tensor("xk_in", [128, KT], dt.float32, kind="ExternalInput").ap()
    yk_in = nc.dram_tensor("yk_in", [128, KT], dt.float32, kind="ExternalInput").ap()
    stT_in = nc.dram_tensor("stT_in", [4, N], dt.float32, kind="ExternalInput").ap()
    w1_in = nc.dram_tensor("w1_in", [4, HID], dt.float32, kind="ExternalInput").ap()
    w2_in = nc.dram_tensor("w2_in", [HID, HID], dt.bfloat16, kind="ExternalInput").ap()
    b1_in = nc.dram_tensor("b1_in", [HID, 1], dt.float32, kind="ExternalInput").ap()
    b2_in = nc.dram_tensor("b2_in", [HID, 1], dt.float32, kind="ExternalInput").ap()
    cw_in = nc.dram_tensor("cw_in", [HID, 1], dt.float32, kind="ExternalInput").ap()
    cb_in = nc.dram_tensor("cb_in", [1, 1], dt.float32, kind="ExternalInput").ap()
    id64_in = nc.dram_tensor("id64_in", [HID, HID], dt.float32, kind="ExternalInput").ap()

    mask_out = nc.dram_tensor("mask_cols", [N, MBLK], dt.uint8, kind="ExternalOutput").ap()
    out_row = nc.dram_tensor("out_row", [1, MBLK], dt.float32, kind="ExternalOutput").ap()

    with tile.TileContext(nc) as tc:
        with (
            tc.tile_pool(name="const", bufs=1) as cpool,
            tc.tile_pool(name="big", bufs=1) as big,
            tc.tile_pool(name="work", bufs=2) as work,
            tc.tile_pool(name="tmp", bufs=1) as tmp,
            tc.tile_pool(name="dram", bufs=1, space="DRAM") as dram,
        ):
            # ---- load constants (xm/ym split over 4 DMAs to parallelize) --
            xm = cpool.tile([128, MBLK], dt.float32, name="xm")
            ym = cpool.tile([128, MBLK], dt.float32, name="ym")
            for q in range(4):
                sl = slice(q * MBLK // 4, (q + 1) * MBLK // 4)
                nc.sync.dma_start(xm[:, sl], xm_in[:, sl])
                nc.sync.dma_start(ym[:, sl], ym_in[:, sl])
            xk = cpool.tile_from(xk_in)            # [128, 64] f32
            yk = cpool.tile_from(yk_in)
            id64 = cpool.tile_from(id64_in)        # [64, 64] f32 identity
            w1 = cpool.tile_from(w1_in)            # [4, 64] f32
            w2 = cpool.tile_from(w2_in)            # [64, 64] bf16
            b1 = cpool.tile_from(b1_in)            # [64, 1] f32
            b2 = cpool.tile_from(b2_in)
            cw = cpool.tile_from(cw_in)
            cb = cpool.tile_from(cb_in)

            ones8 = cpool.tile([128, 1], dt.float8e4)
            nc.gpsimd.memset(ones8[:], 1.0)
            ones32 = cpool.tile([1, HID], dt.float32)
            nc.gpsimd.memset(ones32[:], 1.0)

            # ---- persistent big tensors ----
            mask8 = big.tile([128, KT * MBLK], dt.float8e4, name="mask8")   # 64 KB/p
            xw1q = big.tile([128, KT * HID], dt.bfloat16, name="xw1q")      # 8 KB/p
            xw2q = big.tile([128, KT * HID], dt.bfloat16, name="xw2q")      # 8 KB/p
            h1full = big.tile([HID, N], dt.bfloat16, name="h1full")         # 16 KB/p
            dinvk = big.tile([128, KT], dt.float32, name="dinvk")
            dinvbc = big.tile([HID, MBLK], dt.float32, name="dinvbc")

            # ================= PHASE 1: mask + deg (+ XW1 on PE) =========
            ph1 = tc.alloc_tile_pool(name="ph1", bufs=1)
            stT = ph1.tile_from(stT_in)                          # [4, 8192] f32
            xw1f = ph1.tile([128, KT * HID], dt.bfloat16, name="xw1f")  # 8 KB/p
            with tc.tile_pool(name="p1ps", bufs=1, space="PSUM") as p1ps:
                degps = p1ps.tile([1, MBLK], dt.float32, name="degps")

                for t in range(KT):
                    sqx = work.tile([128, MBLK], dt.float32, tag="sqx")
                    sqy = work.tile([128, MBLK], dt.float32, tag="sqy")
                    d2 = work.tile([128, MBLK], dt.float32, tag="d2")
                    # (x_k - x_m)^2 : exact fp32 (scale=-1 mult is exact,
                    # then one rounded add, then Square = one rounded mult)
                    if t % 7 == 3:
                        # ACT/DVE load balance: do the x square on DVE
                        dxt = work.tile([128, MBLK], dt.float32, tag="d2")
                        nc.vector.tensor_scalar(dxt[:], xm[:], xk[:, t:t + 1],
                                                None, Alu.subtract)
                        nc.vector.tensor_tensor(sqx[:], dxt[:], dxt[:], Alu.mult)
                    else:
                        nc.scalar.activation(sqx[:], xm[:], Act.Square,
                                             bias=xk[:, t:t + 1], scale=-1.0)
                    nc.scalar.activation(sqy[:], ym[:], Act.Square,
                                         bias=yk[:, t:t + 1], scale=-1.0)
                    nc.vector.tensor_tensor(d2[:], sqx[:], sqy[:], Alu.add)
                    # mask tile in fp8 (0.0 / 1.0); doubles as the bool
                    # output (raw bytes 0x00/0x38, host decodes with != 0)
                    msl = mask8[:, t * MBLK:(t + 1) * MBLK]
                    nc.vector.tensor_scalar(msl, d2[:], OBS_R2, None, Alu.is_le)
                    nc.gpsimd.dma_start(mask_out[t * 128:(t + 1) * 128, :],
                                        msl.bitcast(dt.uint8))
                    # deg += ones^T @ mask_tile  (exact integer sums in fp32)
                    for h in range(2):
                        nc.tensor.matmul(
                            degps[:, h * 512:(h + 1) * 512],
                            ones8[:],
                            msl[:, h * 512:(h + 1) * 512],
                            start=(t == 0), stop=(t == KT - 1))

                # XW1[k, c] = states @ W1 (fp32 PE), packed 8 k-tiles per bank
                with tc.tile_pool(name="xwps", bufs=2, space="PSUM") as xwps:
                    for g in range(KT // 8):
                        ps = xwps.tile([128, 512], dt.float32, tag="xw")
                        for j in range(8):
                            t = g * 8 + j
                            nc.tensor.matmul(
                                ps[:, j * HID:(j + 1) * HID],
                                stT[:, t * 128:(t + 1) * 128],
                                w1[:], start=True, stop=True)
                        nc.scalar.copy(xw1f[:, g * 512:(g + 1) * 512], ps[:])

                # ---- dinv = deg^-1/2 = exp(-0.5 * ln(deg)) (deg is exact) --
                lgdeg = tmp.tile([1, MBLK], dt.float32, tag="sm")
                dinvm = cpool.tile([1, MBLK], dt.float32)
                nc.scalar.activation(lgdeg[:], degps[:], Act.Ln)
                nc.scalar.activation(dinvm[:], lgdeg[:], Act.Exp, scale=-0.5)

            # ---- AllGather dinv across the 8 cores ----
            dinv_src = dram.tile([1, MBLK], dt.float32, name="dinv_src")
            dinv_all = dram.tile([NCORES, MBLK], dt.float32, name="dinv_all",
                                 addr_space="Shared")
            nc.sync.dma_start(dinv_src[:], dinvm[:])
            nc.gpsimd.collective_compute(
                "AllGather", Alu.bypass,
                replica_groups=[list(range(NCORES))],
                ins=[dinv_src[:].opt()], outs=[dinv_all[:].opt()])
            # dinvk[p, t] = dinv[t*128 + p]: contiguous [64,128] load, then
            # PE transpose (a direct strided gather DMA costs ~11us)
            dinvT = tmp.tile([KT, 128], dt.float32, tag="dinvT")
            nc.sync.dma_start(
                dinvT[:],
                dinv_all[:].rearrange("a b -> (a b)").rearrange("(t p) -> t p", p=128))
            with tc.tile_pool(name="trps", bufs=1, space="PSUM") as trps:
                dkps = trps.tile([128, KT], dt.float32, name="dkps")
                nc.tensor.transpose(dkps[:], dinvT[:], id64[:])
                nc.scalar.copy(dinvk[:], dkps[:])

            # dinvbc[c, m] = dinv_m[m] broadcast over 64 partitions (PE trick)
            with tc.tile_pool(name="bcps", bufs=1, space="PSUM") as bcps:
                dbc = bcps.tile([HID, MBLK], dt.float32, name="dbc")
                for h in range(2):
                    nc.tensor.matmul(dbc[:, h * 512:(h + 1) * 512], ones32[:],
                                     dinvm[:, h * 512:(h + 1) * 512],
                                     start=True, stop=True)
                nc.scalar.copy(dinvbc[:], dbc[:])

            # XW1' = dinv[k] * XW1[k, :]  -> fp8
            for t in range(KT):
                nc.vector.tensor_scalar(
                    xw1q[:, t * HID:(t + 1) * HID],
                    xw1f[:, t * HID:(t + 1) * HID],
                    dinvk[:, t:t + 1], None, Alu.mult)
            ph1.release()

            # ================= LAYER 1 SpMM =================
            h1p = cpool.tile([HID, MBLK], dt.bfloat16)
            with tc.tile_pool(name="aggps", bufs=1, space="PSUM") as aggps:
                agg = aggps.tile([HID, MBLK], dt.float32, name="agg1")
                for t in range(KT):
                    for h in range(2):
                        nc.tensor.matmul(
                            agg[:, h * 512:(h + 1) * 512],
                            xw1q[:, t * HID:(t + 1) * HID],
                            mask8[:, t * MBLK + h * 512: t * MBLK + (h + 1) * 512],
                            start=(t == 0), stop=(t == KT - 1))
                # h1 = relu(dinv_m * agg + b1);  h1' = dinv_m * h1 (bf16)
                t1 = tmp.tile([HID, MBLK], dt.float32, tag="tt")
                h1T = tmp.tile([HID, MBLK], dt.float32, tag="hh")
                nc.vector.tensor_tensor(t1[:], agg[:], dinvbc[:], Alu.mult)
                nc.scalar.activation(h1T[:], t1[:], Act.Relu, bias=b1[:])
                nc.vector.tensor_tensor(h1p[:], h1T[:], dinvbc[:], Alu.mult)

            # ---- AllGather h1' ----
            h1_src = dram.tile([HID, MBLK], dt.bfloat16, name="h1_src")
            h1_all = dram.tile([NCORES * HID, MBLK], dt.bfloat16, name="h1_all",
                               addr_space="Shared")
            nc.sync.dma_start(h1_src[:], h1p[:])
            nc.gpsimd.collective_compute(
                "AllGather", Alu.bypass,
                replica_groups=[list(range(NCORES))],
                ins=[h1_src[:].opt()], outs=[h1_all[:].opt()])
            nc.sync.dma_start(
                h1full[:].rearrange("p (r m) -> p r m", r=NCORES),
                h1_all[:].rearrange("(r p) m -> p r m", p=HID))

            # XW2' = h1'^T stationary @ W2 (bf16) -> fp8 (dinv already folded)
            with tc.tile_pool(name="xwps2", bufs=2, space="PSUM") as xwps2:
                for g in range(KT // 8):
                    ps = xwps2.tile([128, 512], dt.float32, tag="xw2")
                    for j in range(8):
                        t = g * 8 + j
                        nc.tensor.matmul(
                            ps[:, j * HID:(j + 1) * HID],
                            h1full[:, t * 128:(t + 1) * 128],
                            w2[:], start=True, stop=True)
                    nc.scalar.copy(xw2q[:, g * 512:(g + 1) * 512], ps[:])

            # ================= LAYER 2 SpMM =================
            h2T = tmp.tile([HID, MBLK], dt.float32, tag="hh")
            with tc.tile_pool(name="aggps2", bufs=1, space="PSUM") as aggps2:
                agg2 = aggps2.tile([HID, MBLK], dt.float32, name="agg2")
                for t in range(KT):
                    for h in range(2):
                        nc.tensor.matmul(
                            agg2[:, h * 512:(h + 1) * 512],
                            xw2q[:, t * HID:(t + 1) * HID],
                            mask8[:, t * MBLK + h * 512: t * MBLK + (h + 1) * 512],
                            start=(t == 0), stop=(t == KT - 1))
                t2 = tmp.tile([HID, MBLK], dt.float32, tag="tt")
                nc.vector.tensor_tensor(t2[:], agg2[:], dinvbc[:], Alu.mult)
                nc.scalar.activation(h2T[:], t2[:], Act.Relu, bias=b2[:])

            # ================= output head =================
            with tc.tile_pool(name="outps", bufs=1, space="PSUM") as outps:
                ops = outps.tile([1, MBLK], dt.float32, name="ops")
                for h in range(2):
                    nc.tensor.matmul(ops[:, h * 512:(h + 1) * 512], cw[:],
                                     h2T[:, h * 512:(h + 1) * 512],
                                     start=True, stop=True)
                osb = tmp.tile([1, MBLK], dt.float32, tag="sm")
                nc.scalar.activation(osb[:], ops[:], Act.Identity, bias=cb[:])
                nc.sync.dma_start(out_row[:], osb[:])

    nc.compile()
    return nc


def _prep_inputs(states, W1, b1, W2, b2, cw, cb):
    states = np.asarray(states, dtype=np.float32)
    x = np.ascontiguousarray(states[:, 0])
    y = np.ascontiguousarray(states[:, 1])
    bf16 = mybir.dt.np(dt.bfloat16)

    xk = np.ascontiguousarray(x.reshape(KT, 128).T)   # [128, 64]
    yk = np.ascontiguousarray(y.reshape(KT, 128).T)
    stT = np.ascontiguousarray(states.T)              # [4, 8192]
    w1 = np.asarray(W1, dtype=np.float32)
    w2 = np.asarray(W2, dtype=np.float32).astype(bf16)
    b1c = np.asarray(b1, dtype=np.float32).reshape(HID, 1)
    b2c = np.asarray(b2, dtype=np.float32).reshape(HID, 1)
    cwc = np.asarray(cw, dtype=np.float32).reshape(HID, 1)
    cbc = np.asarray(cb, dtype=np.float32).reshape(1, 1)
    id64 = np.eye(HID, dtype=np.float32)

    in_maps = []
    for c in range(NCORES):
        xm = np.ascontiguousarray(
            np.broadcast_to(x[c * MBLK:(c + 1) * MBLK][None, :], (128, MBLK)))
        ym = np.ascontiguousarray(
            np.broadcast_to(y[c * MBLK:(c + 1) * MBLK][None, :], (128, MBLK)))
        in_maps.append({
            "xm_in": xm, "ym_in": ym, "xk_in": xk, "yk_in": yk,
            "stT_in": stT, "w1_in": w1, "w2_in": w2,
            "b1_in": b1c, "b2_in": b2c, "cw_in": cwc, "cb_in": cbc,
            "id64_in": id64,
        })
    return in_maps


def _get_nc():
    if "nc" not in _CACHE:
        _CACHE["nc"] = build_nc()
    return _CACHE["nc"]


def _install_ntff_hook():
    """Recreate the antenv.axon_hooks shim this image is missing."""
    import sys, types
    try:
        from antenv.axon_hooks import get_axon_ntff_profile_hook  # noqa
        return
    except ImportError:
        pass
    try:
        import antenv
        sys.path.insert(0, "/root/.axon_site/trn_agent_boot")
        import trn_boot
        hook = trn_boot._ntff_profile_via_ctypes("/opt/axon/libaxon_pjrt.so")
        mod = types.ModuleType("antenv.axon_hooks")
        mod._hook = hook
        mod.get_axon_ntff_profile_hook = lambda: mod._hook
        mod.set_axon_ntff_profile_hook = lambda h: setattr(mod, "_hook", h)
        sys.modules["antenv.axon_hooks"] = mod
        antenv.axon_hooks = mod
    except Exception as e:  # tracing is best-effort
        print("ntff hook install failed:", e)


def run(trace=False, **inputs):
    if trace:
        _install_ntff_hook()
    nc = _get_nc()
    in_maps = _prep_inputs(**inputs)
    res = bass_utils.run_bass_kernel_spmd(
        nc, in_maps, core_ids=list(range(NCORES)), trace=trace)
    mask = np.concatenate([r["mask_cols"] for r in res.results], axis=1)
    out = np.concatenate(
        [r["out_row"].reshape(MBLK, 1) for r in res.results], axis=0)
    return (out.astype(np.float32), mask.astype(bool)), res


def kernel(**inputs):
    (out, mask), _ = run(trace=False, **inputs)
    return out, mask


# revision 36
# speedup vs baseline: 1.3435x; 1.1172x over previous
"""Trainium2 Bass kernel for nn_CBF (GCN message passing over a radius graph).

8-core SPMD: core c owns agent block m in [c*1024, (c+1)*1024).
Each core builds its [8192 x 1024] column strip of the symmetric adjacency
mask exactly (elementwise fp32, matching jnp rounding), aggregates with the
normalized adjacency via fp8 matmuls on the PE, and all-gathers the small
[N,64] feature matrix between the two GCN layers.

Outputs per core: mask_cols [8192, 1024] u8 (full mask = concat axis=1,
valid because the mask is symmetric... actually because the tiles are
row-indexed by the global k axis directly), out_row [1, 1024] f32.
"""

import numpy as np

import concourse.bass as bass
import concourse.bacc as bacc
import concourse.mybir as mybir
import concourse.tile as tile
from concourse import bass_utils

dt = mybir.dt
Alu = mybir.AluOpType
Act = mybir.ActivationFunctionType

N = 8192
NCORES = 8
MBLK = N // NCORES          # 1024 agents per core
KT = N // 128               # 64 k-tiles of 128 partitions
HID = 64
OBS_R2 = 1.0

_CACHE = {}


def build_nc():
    nc = bacc.Bacc("TRN2", target_bir_lowering=False, debug=False,
                   num_devices=NCORES)

    # ---- kernel I/O (per-core) ----
    xm_in = nc.dram_tensor("xm_in", [128, MBLK], dt.float32, kind="ExternalInput").ap()
    ym_in = nc.dram_tensor("ym_in", [128, MBLK], dt.float32, kind="ExternalInput").ap()
    xk_in = nc.dram_tensor("xk_in", [128, KT], dt.float32, kind="ExternalInput").ap()
    yk_in = nc.dram_tensor("yk_in", [128, KT], dt.float32, kind="ExternalInput").ap()
    stT_in = nc.dram_tensor("stT_in", [4, N], dt.float32, kind="ExternalInput").ap()
    w1_in = nc.dram_tensor("w1_in", [4, HID], dt.float32, kind="ExternalInput").ap()
    w2_in = nc.dram_tensor("w2_in", [HID, HID], dt.bfloat16, kind="ExternalInput").ap()
    b1_in = nc.dram_tensor("b1_in", [HID, 1], dt.float32, kind="ExternalInput").ap()
    b2_in = nc.dram_tensor("b2_in", [HID, 1], dt.float32, kind="ExternalInput").ap()
    cw_in = nc.dram_tensor("cw_in", [HID, 1], dt.float32, kind="ExternalInput").ap()
    cb_in = nc.dram_tensor("cb_in", [1, 1], dt.float32, kind="ExternalInput").ap()
    id64_in = nc.dram_tensor("id64_in", [HID, HID], dt.float32, kind="ExternalInput").ap()

    mask_out = nc.dram_tensor("mask_cols", [N, MBLK], dt.uint8, kind="ExternalOutput").ap()
    out_row = nc.dram_tensor("out_row", [1, MBLK], dt.float32, kind="ExternalOutput").ap()

    with tile.TileContext(nc) as tc:
        with (
            tc.tile_pool(name="const", bufs=1) as cpool,
            tc.tile_pool(name="big", bufs=1) as big,
            tc.tile_pool(name="work", bufs=3) as work,
            tc.tile_pool(name="tmp", bufs=1) as tmp,
            tc.tile_pool(name="dram", bufs=1, space="DRAM") as dram,
        ):
            # ---- load constants (xm/ym split over 4 DMAs to parallelize) --
            xm = cpool.tile([128, MBLK], dt.float32, name="xm")
            ym = cpool.tile([128, MBLK], dt.float32, name="ym")
            for q in range(4):
                sl = slice(q * MBLK // 4, (q + 1) * MBLK // 4)
                nc.sync.dma_start(xm[:, sl], xm_in[:, sl])
                nc.sync.dma_start(ym[:, sl], ym_in[:, sl])
            xk = cpool.tile_from(xk_in)            # [128, 64] f32
            yk = cpool.tile_from(yk_in)
            id64 = cpool.tile_from(id64_in)        # [64, 64] f32 identity
            w1 = cpool.tile_from(w1_in)            # [4, 64] f32
            w2 = cpool.tile_from(w2_in)            # [64, 64] bf16
            b1 = cpool.tile_from(b1_in)            # [64, 1] f32
            b2 = cpool.tile_from(b2_in)
            cw = cpool.tile_from(cw_in)
            cb = cpool.tile_from(cb_in)

            ones8 = cpool.tile([128, 1], dt.float8e4)
            nc.gpsimd.memset(ones8[:], 1.0)
            ones32 = cpool.tile([1, HID], dt.float32)
            nc.gpsimd.memset(ones32[:], 1.0)

            # Dummy collective at t~0: prepays the ~11.5us first-collective
            # ncfw warmup so the real dinv AllGather starts promptly.
            warm_in = dram.tile([1, 8], dt.float32, name="warm_in")
            warm_out = dram.tile([NCORES, 8], dt.float32, name="warm_out",
                                 addr_space="Shared")
            wtile = cpool.tile([1, 8], dt.float32)
            nc.gpsimd.memset(wtile[:], 0.0)
            nc.sync.dma_start(warm_in[:], wtile[:])
            nc.gpsimd.collective_compute(
                "AllGather", Alu.bypass,
                replica_groups=[list(range(NCORES))],
                ins=[warm_in[:].opt()], outs=[warm_out[:].opt()])

            # ---- persistent big tensors ----
            mask8 = big.tile([128, KT * MBLK], dt.float8e4, name="mask8")   # 64 KB/p
            xw1q = big.tile([128, KT * HID], dt.bfloat16, name="xw1q")      # 8 KB/p
            xw2q = big.tile([128, KT * HID], dt.bfloat16, name="xw2q")      # 8 KB/p
            h1full = big.tile([HID, N], dt.bfloat16, name="h1full")         # 16 KB/p
            dinvk = big.tile([128, KT], dt.float32, name="dinvk")
            dinvbc = big.tile([HID, MBLK], dt.float32, name="dinvbc")

            # ================= PHASE 1: mask + deg (+ XW1 on PE) =========
            ph1 = tc.alloc_tile_pool(name="ph1", bufs=1)
            stT = ph1.tile_from(stT_in)                          # [4, 8192] f32
            xw1f = ph1.tile([128, KT * HID], dt.bfloat16, name="xw1f")  # 8 KB/p
            with tc.tile_pool(name="p1ps", bufs=1, space="PSUM") as p1ps:
                degps = p1ps.tile([1, MBLK], dt.float32, name="degps")

                for t in range(KT):
                    sqx = work.tile([128, MBLK], dt.float32, tag="sqx")
                    sqy = work.tile([128, MBLK], dt.float32, tag="sqy")
                    d2 = work.tile([128, MBLK], dt.float32, tag="d2")
                    # (x_k - x_m)^2 : exact fp32 (scale=-1 mult is exact,
                    # then one rounded add, then Square = one rounded mult)
                    if t % 7 == 3:
                        # ACT/DVE load balance: do the x square on DVE
                        dxt = work.tile([128, MBLK], dt.float32, tag="d2")
                        nc.vector.tensor_scalar(dxt[:], xm[:], xk[:, t:t + 1],
                                                None, Alu.subtract)
                        nc.vector.tensor_tensor(sqx[:], dxt[:], dxt[:], Alu.mult)
                    else:
                        nc.scalar.activation(sqx[:], xm[:], Act.Square,
                                             bias=xk[:, t:t + 1], scale=-1.0)
                    nc.scalar.activation(sqy[:], ym[:], Act.Square,
                                         bias=yk[:, t:t + 1], scale=-1.0)
                    nc.vector.tensor_tensor(d2[:], sqx[:], sqy[:], Alu.add)
                    # mask tile in fp8 (0.0 / 1.0); doubles as the bool
                    # output (raw bytes 0x00/0x38, host decodes with != 0)
                    msl = mask8[:, t * MBLK:(t + 1) * MBLK]
                    nc.vector.tensor_scalar(msl, d2[:], OBS_R2, None, Alu.is_le)
                    nc.gpsimd.dma_start(mask_out[t * 128:(t + 1) * 128, :],
                                        msl.bitcast(dt.uint8))
                    # deg += ones^T @ mask_tile  (exact integer sums in fp32)
                    for h in range(2):
                        nc.tensor.matmul(
                            degps[:, h * 512:(h + 1) * 512],
                            ones8[:],
                            msl[:, h * 512:(h + 1) * 512],
                            start=(t == 0), stop=(t == KT - 1))

                # XW1[k, c] = states @ W1 (fp32 PE), packed 8 k-tiles per bank
                with tc.tile_pool(name="xwps", bufs=2, space="PSUM") as xwps:
                    for g in range(KT // 8):
                        ps = xwps.tile([128, 512], dt.float32, tag="xw")
                        for j in range(8):
                            t = g * 8 + j
                            nc.tensor.matmul(
                                ps[:, j * HID:(j + 1) * HID],
                                stT[:, t * 128:(t + 1) * 128],
                                w1[:], start=True, stop=True)
                        nc.scalar.copy(xw1f[:, g * 512:(g + 1) * 512], ps[:])

                # ---- dinv = deg^-1/2 = exp(-0.5 * ln(deg)) (deg is exact) --
                lgdeg = tmp.tile([1, MBLK], dt.float32, tag="sm")
                dinvm = cpool.tile([1, MBLK], dt.float32)
                nc.scalar.activation(lgdeg[:], degps[:], Act.Ln)
                nc.scalar.activation(dinvm[:], lgdeg[:], Act.Exp, scale=-0.5)

            # ---- AllGather dinv across the 8 cores ----
            dinv_src = dram.tile([1, MBLK], dt.float32, name="dinv_src")
            dinv_all = dram.tile([NCORES, MBLK], dt.float32, name="dinv_all",
                                 addr_space="Shared")
            nc.sync.dma_start(dinv_src[:], dinvm[:])
            nc.gpsimd.collective_compute(
                "AllGather", Alu.bypass,
                replica_groups=[list(range(NCORES))],
                ins=[dinv_src[:].opt()], outs=[dinv_all[:].opt()])
            # dinvk[p, t] = dinv[t*128 + p]: contiguous [64,128] load, then
            # PE transpose (a direct strided gather DMA costs ~11us)
            dinvT = tmp.tile([KT, 128], dt.float32, tag="dinvT")
            nc.sync.dma_start(
                dinvT[:],
                dinv_all[:].rearrange("a b -> (a b)").rearrange("(t p) -> t p", p=128))
            with tc.tile_pool(name="trps", bufs=1, space="PSUM") as trps:
                dkps = trps.tile([128, KT], dt.float32, name="dkps")
                nc.tensor.transpose(dkps[:], dinvT[:], id64[:])
                nc.scalar.copy(dinvk[:], dkps[:])

            # dinvbc[c, m] = dinv_m[m] broadcast over 64 partitions (PE trick)
            with tc.tile_pool(name="bcps", bufs=1, space="PSUM") as bcps:
                dbc = bcps.tile([HID, MBLK], dt.float32, name="dbc")
                for h in range(2):
                    nc.tensor.matmul(dbc[:, h * 512:(h + 1) * 512], ones32[:],
                                     dinvm[:, h * 512:(h + 1) * 512],
                                     start=True, stop=True)
                nc.scalar.copy(dinvbc[:], dbc[:])

            # XW1' = dinv[k] * XW1[k, :]  -> fp8
            for t in range(KT):
                nc.vector.tensor_scalar(
                    xw1q[:, t * HID:(t + 1) * HID],
                    xw1f[:, t * HID:(t + 1) * HID],
                    dinvk[:, t:t + 1], None, Alu.mult)
            ph1.release()

            # ================= LAYER 1 SpMM =================
            h1p = cpool.tile([HID, MBLK], dt.bfloat16)
            with tc.tile_pool(name="aggps", bufs=1, space="PSUM") as aggps:
                agg = aggps.tile([HID, MBLK], dt.float32, name="agg1")
                for t in range(KT):
                    for h in range(2):
                        nc.tensor.matmul(
                            agg[:, h * 512:(h + 1) * 512],
                            xw1q[:, t * HID:(t + 1) * HID],
                            mask8[:, t * MBLK + h * 512: t * MBLK + (h + 1) * 512],
                            start=(t == 0), stop=(t == KT - 1))
                # h1 = relu(dinv_m * agg + b1);  h1' = dinv_m * h1 (bf16)
                t1 = tmp.tile([HID, MBLK], dt.float32, tag="tt")
                h1T = tmp.tile([HID, MBLK], dt.float32, tag="hh")
                nc.vector.tensor_tensor(t1[:], agg[:], dinvbc[:], Alu.mult)
                nc.scalar.activation(h1T[:], t1[:], Act.Relu, bias=b1[:])
                nc.vector.tensor_tensor(h1p[:], h1T[:], dinvbc[:], Alu.mult)

            # ---- AllGather h1' ----
            h1_src = dram.tile([HID, MBLK], dt.bfloat16, name="h1_src")
            h1_all = dram.tile([NCORES * HID, MBLK], dt.bfloat16, name="h1_all",
                               addr_space="Shared")
            nc.sync.dma_start(h1_src[:], h1p[:])
            nc.gpsimd.collective_compute(
                "AllGather", Alu.bypass,
                replica_groups=[list(range(NCORES))],
                ins=[h1_src[:].opt()], outs=[h1_all[:].opt()])
            for r in range(NCORES):
                nc.sync.dma_start(
                    h1full[:, r * MBLK:(r + 1) * MBLK],
                    h1_all[r * HID:(r + 1) * HID, :])

            # XW2' = h1'^T stationary @ W2 (bf16) -> fp8 (dinv already folded)
            with tc.tile_pool(name="xwps2", bufs=2, space="PSUM") as xwps2:
                for g in range(KT // 8):
                    ps = xwps2.tile([128, 512], dt.float32, tag="xw2")
                    for j in range(8):
                        t = g * 8 + j
                        nc.tensor.matmul(
                            ps[:, j * HID:(j + 1) * HID],
                            h1full[:, t * 128:(t + 1) * 128],
                            w2[:], start=True, stop=True)
                    nc.scalar.copy(xw2q[:, g * 512:(g + 1) * 512], ps[:])

            # ================= LAYER 2 SpMM =================
            h2T = tmp.tile([HID, MBLK], dt.float32, tag="hh")
            with tc.tile_pool(name="aggps2", bufs=1, space="PSUM") as aggps2:
                agg2 = aggps2.tile([HID, MBLK], dt.float32, name="agg2")
                for t in range(KT):
                    for h in range(2):
                        nc.tensor.matmul(
                            agg2[:, h * 512:(h + 1) * 512],
                            xw2q[:, t * HID:(t + 1) * HID],
                            mask8[:, t * MBLK + h * 512: t * MBLK + (h + 1) * 512],
                            start=(t == 0), stop=(t == KT - 1))
                t2 = tmp.tile([HID, MBLK], dt.float32, tag="tt")
                nc.vector.tensor_tensor(t2[:], agg2[:], dinvbc[:], Alu.mult)
                nc.scalar.activation(h2T[:], t2[:], Act.Relu, bias=b2[:])

            # ================= output head =================
            with tc.tile_pool(name="outps", bufs=1, space="PSUM") as outps:
                ops = outps.tile([1, MBLK], dt.float32, name="ops")
                for h in range(2):
                    nc.tensor.matmul(ops[:, h * 512:(h + 1) * 512], cw[:],
                                     h2T[:, h * 512:(h + 1) * 512],
                                     start=True, stop=True)
                osb = tmp.tile([1, MBLK], dt.float32, tag="sm")
                nc.scalar.activation(osb[:], ops[:], Act.Identity, bias=cb[:])
                nc.sync.dma_start(out_row[:], osb[:])

    nc.compile()
    return nc


def _prep_inputs(states, W1, b1, W2, b2, cw, cb):
    states = np.asarray(states, dtype=np.float32)
    x = np.ascontiguousarray(states[:, 0])
    y = np.ascontiguousarray(states[:, 1])
    bf16 = mybir.dt.np(dt.bfloat16)

    xk = np.ascontiguousarray(x.reshape(KT, 128).T)   # [128, 64]
    yk = np.ascontiguousarray(y.reshape(KT, 128).T)
    stT = np.ascontiguousarray(states.T)              # [4, 8192]
    w1 = np.asarray(W1, dtype=np.float32)
    w2 = np.asarray(W2, dtype=np.float32).astype(bf16)
    b1c = np.asarray(b1, dtype=np.float32).reshape(HID, 1)
    b2c = np.asarray(b2, dtype=np.float32).reshape(HID, 1)
    cwc = np.asarray(cw, dtype=np.float32).reshape(HID, 1)
    cbc = np.asarray(cb, dtype=np.float32).reshape(1, 1)
    id64 = np.eye(HID, dtype=np.float32)

    in_maps = []
    for c in range(NCORES):
        xm = np.ascontiguousarray(
            np.broadcast_to(x[c * MBLK:(c + 1) * MBLK][None, :], (128, MBLK)))
        ym = np.ascontiguousarray(
            np.broadcast_to(y[c * MBLK:(c + 1) * MBLK][None, :], (128, MBLK)))
        in_maps.append({
            "xm_in": xm, "ym_in": ym, "xk_in": xk, "yk_in": yk,
            "stT_in": stT, "w1_in": w1, "w2_in": w2,
            "b1_in": b1c, "b2_in": b2c, "cw_in": cwc, "cb_in": cbc,
            "id64_in": id64,
        })
    return in_maps


def _get_nc():
    if "nc" not in _CACHE:
        _CACHE["nc"] = build_nc()
    return _CACHE["nc"]


def _install_ntff_hook():
    """Recreate the antenv.axon_hooks shim this image is missing."""
    import sys, types
    try:
        from antenv.axon_hooks import get_axon_ntff_profile_hook  # noqa
        return
    except ImportError:
        pass
    try:
        import antenv
        sys.path.insert(0, "/root/.axon_site/trn_agent_boot")
        import trn_boot
        hook = trn_boot._ntff_profile_via_ctypes("/opt/axon/libaxon_pjrt.so")
        mod = types.ModuleType("antenv.axon_hooks")
        mod._hook = hook
        mod.get_axon_ntff_profile_hook = lambda: mod._hook
        mod.set_axon_ntff_profile_hook = lambda h: setattr(mod, "_hook", h)
        sys.modules["antenv.axon_hooks"] = mod
        antenv.axon_hooks = mod
    except Exception as e:  # tracing is best-effort
        print("ntff hook install failed:", e)


def run(trace=False, **inputs):
    if trace:
        _install_ntff_hook()
    nc = _get_nc()
    in_maps = _prep_inputs(**inputs)
    res = bass_utils.run_bass_kernel_spmd(
        nc, in_maps, core_ids=list(range(NCORES)), trace=trace)
    mask = np.concatenate([r["mask_cols"] for r in res.results], axis=1)
    out = np.concatenate(
        [r["out_row"].reshape(MBLK, 1) for r in res.results], axis=0)
    return (out.astype(np.float32), mask.astype(bool)), res


def kernel(**inputs):
    (out, mask), _ = run(trace=False, **inputs)
    return out, mask


# revision 45
# speedup vs baseline: 1.3826x; 1.0291x over previous
"""Trainium2 Bass kernel for nn_CBF (GCN message passing over a radius graph).

8-core SPMD: core c owns agent block m in [c*1024, (c+1)*1024).
Each core builds its [8192 x 1024] column strip of the symmetric adjacency
mask exactly (elementwise fp32, matching jnp rounding), aggregates with the
normalized adjacency via fp8 matmuls on the PE, and all-gathers the small
[N,64] feature matrix between the two GCN layers.

Outputs per core: mask_cols [8192, 1024] u8 (full mask = concat axis=1,
valid because the mask is symmetric... actually because the tiles are
row-indexed by the global k axis directly), out_row [1, 1024] f32.
"""

import numpy as np

import concourse.bass as bass
import concourse.bacc as bacc
import concourse.mybir as mybir
import concourse.tile as tile
from concourse import bass_utils

dt = mybir.dt
Alu = mybir.AluOpType
Act = mybir.ActivationFunctionType

N = 8192
NCORES = 8
MBLK = N // NCORES          # 1024 agents per core
KT = N // 128               # 64 k-tiles of 128 partitions
HID = 64
OBS_R2 = 1.0

_CACHE = {}


def build_nc():
    nc = bacc.Bacc("TRN2", target_bir_lowering=False, debug=False,
                   num_devices=NCORES)

    # ---- kernel I/O (per-core) ----
    xm_in = nc.dram_tensor("xm_in", [128, MBLK], dt.float32, kind="ExternalInput").ap()
    ym_in = nc.dram_tensor("ym_in", [128, MBLK], dt.float32, kind="ExternalInput").ap()
    xk_in = nc.dram_tensor("xk_in", [128, KT], dt.float32, kind="ExternalInput").ap()
    yk_in = nc.dram_tensor("yk_in", [128, KT], dt.float32, kind="ExternalInput").ap()
    stT_in = nc.dram_tensor("stT_in", [4, N], dt.float32, kind="ExternalInput").ap()
    w1_in = nc.dram_tensor("w1_in", [4, HID], dt.float32, kind="ExternalInput").ap()
    w2_in = nc.dram_tensor("w2_in", [HID, HID], dt.bfloat16, kind="ExternalInput").ap()
    b1_in = nc.dram_tensor("b1_in", [HID, 1], dt.float32, kind="ExternalInput").ap()
    b2_in = nc.dram_tensor("b2_in", [HID, 1], dt.float32, kind="ExternalInput").ap()
    cw_in = nc.dram_tensor("cw_in", [HID, 1], dt.float32, kind="ExternalInput").ap()
    cb_in = nc.dram_tensor("cb_in", [1, 1], dt.float32, kind="ExternalInput").ap()
    id64_in = nc.dram_tensor("id64_in", [HID, HID], dt.float32, kind="ExternalInput").ap()

    mask_out = nc.dram_tensor("mask_cols", [N, MBLK], dt.uint8, kind="ExternalOutput").ap()
    out_row = nc.dram_tensor("out_row", [1, MBLK], dt.float32, kind="ExternalOutput").ap()

    with tile.TileContext(nc) as tc:
        with (
            tc.tile_pool(name="const", bufs=1) as cpool,
            tc.tile_pool(name="big", bufs=1) as big,
            tc.tile_pool(name="work", bufs=3) as work,
            tc.tile_pool(name="tmp", bufs=1) as tmp,
            tc.tile_pool(name="dram", bufs=1, space="DRAM") as dram,
        ):
            # ---- load constants; spread DMA *issues* across engine queues
            # (a dma_start issue costs ~600ns serialized on one queue)
            xm = cpool.tile([128, MBLK], dt.float32, name="xm")
            ym = cpool.tile([128, MBLK], dt.float32, name="ym")
            half = MBLK // 2
            nc.sync.dma_start(xm[:, 0:half], xm_in[:, 0:half])
            nc.scalar.dma_start(xm[:, half:], xm_in[:, half:])
            nc.gpsimd.dma_start(ym[:, 0:half], ym_in[:, 0:half])
            nc.sync.dma_start(ym[:, half:], ym_in[:, half:])
            xk = cpool.tile([128, KT], dt.float32, name="xk")
            yk = cpool.tile([128, KT], dt.float32, name="yk")
            nc.gpsimd.dma_start(xk[:], xk_in[:])
            nc.gpsimd.dma_start(yk[:], yk_in[:])
            id64 = cpool.tile([HID, HID], dt.float32, name="id64")
            w1 = cpool.tile([4, HID], dt.float32, name="w1")
            w2 = cpool.tile([HID, HID], dt.bfloat16, name="w2")
            b1 = cpool.tile([HID, 1], dt.float32, name="b1")
            b2 = cpool.tile([HID, 1], dt.float32, name="b2")
            cw = cpool.tile([HID, 1], dt.float32, name="cw")
            cb = cpool.tile([1, 1], dt.float32, name="cb")
            nc.scalar.dma_start(id64[:], id64_in[:])
            nc.scalar.dma_start(w1[:], w1_in[:])
            nc.sync.dma_start(w2[:], w2_in[:])
            nc.sync.dma_start(b1[:], b1_in[:])
            nc.scalar.dma_start(b2[:], b2_in[:])
            nc.sync.dma_start(cw[:], cw_in[:])
            nc.scalar.dma_start(cb[:], cb_in[:])

            ones8 = cpool.tile([128, 1], dt.float8e4)
            nc.gpsimd.memset(ones8[:], 1.0)
            ones32 = cpool.tile([1, HID], dt.float32)
            nc.gpsimd.memset(ones32[:], 1.0)

            # Dummy collective at t~0: prepays the ~11.5us first-collective
            # ncfw warmup so the real dinv AllGather starts promptly.
            warm_in = dram.tile([1, 8], dt.float32, name="warm_in")
            warm_out = dram.tile([NCORES, 8], dt.float32, name="warm_out",
                                 addr_space="Shared")
            wtile = cpool.tile([1, 8], dt.float32)
            nc.gpsimd.memset(wtile[:], 0.0)
            nc.sync.dma_start(warm_in[:], wtile[:])
            # dummy Square: forces the ACT table load at t~0 instead of
            # right before the first real Square
            dsq = cpool.tile([1, 8], dt.float32)
            nc.scalar.activation(dsq[:], wtile[:], Act.Square)
            nc.gpsimd.collective_compute(
                "AllGather", Alu.bypass,
                replica_groups=[list(range(NCORES))],
                ins=[warm_in[:].opt()], outs=[warm_out[:].opt()])

            # ---- persistent big tensors ----
            mask8 = big.tile([128, KT * MBLK], dt.float8e4, name="mask8")   # 64 KB/p
            xw1q = big.tile([128, KT * HID], dt.bfloat16, name="xw1q")      # 8 KB/p
            xw2q = big.tile([128, KT * HID], dt.bfloat16, name="xw2q")      # 8 KB/p
            h1full = big.tile([HID, N], dt.bfloat16, name="h1full")         # 16 KB/p
            dinvk = big.tile([128, KT], dt.float32, name="dinvk")
            dinvbc = big.tile([HID, MBLK], dt.float32, name="dinvbc")

            # ================= PHASE 1: mask + deg (+ XW1 on PE) =========
            ph1 = tc.alloc_tile_pool(name="ph1", bufs=1)
            stT = ph1.tile_from(stT_in)                          # [4, 8192] f32
            xw1f = ph1.tile([128, KT * HID], dt.bfloat16, name="xw1f")  # 8 KB/p
            with tc.tile_pool(name="p1ps", bufs=1, space="PSUM") as p1ps:
                degps = p1ps.tile([1, MBLK], dt.float32, name="degps")

                for t in range(KT):
                    sqx = work.tile([128, MBLK], dt.float32, tag="sqx")
                    sqy = work.tile([128, MBLK], dt.float32, tag="sqy")
                    d2 = work.tile([128, MBLK], dt.float32, tag="d2")
                    # (x_k - x_m)^2 : exact fp32 (scale=-1 mult is exact,
                    # then one rounded add, then Square = one rounded mult)
                    if t % 7 == 3:
                        # ACT/DVE load balance: do the x square on DVE
                        dxt = work.tile([128, MBLK], dt.float32, tag="d2")
                        nc.vector.tensor_scalar(dxt[:], xm[:], xk[:, t:t + 1],
                                                None, Alu.subtract)
                        nc.vector.tensor_tensor(sqx[:], dxt[:], dxt[:], Alu.mult)
                    else:
                        nc.scalar.activation(sqx[:], xm[:], Act.Square,
                                             bias=xk[:, t:t + 1], scale=-1.0)
                    nc.scalar.activation(sqy[:], ym[:], Act.Square,
                                         bias=yk[:, t:t + 1], scale=-1.0)
                    nc.vector.tensor_tensor(d2[:], sqx[:], sqy[:], Alu.add)
                    # mask tile in fp8 (0.0 / 1.0); doubles as the bool
                    # output (raw bytes 0x00/0x38, host decodes with != 0)
                    msl = mask8[:, t * MBLK:(t + 1) * MBLK]
                    nc.vector.tensor_scalar(msl, d2[:], OBS_R2, None, Alu.is_le)
                    if t % 4 == 3:
                        # batched mask write-out: 4 k-tiles per DMA
                        g0 = t - 3
                        src = mask8[:, g0 * MBLK:(t + 1) * MBLK].bitcast(dt.uint8)
                        nc.gpsimd.dma_start(
                            mask_out[g0 * 128:(t + 1) * 128, :].rearrange(
                                "(j p) m -> p j m", p=128),
                            src.rearrange("p (j m) -> p j m", j=4))
                    # deg += ones^T @ mask_tile  (exact integer sums in fp32)
                    for h in range(2):
                        nc.tensor.matmul(
                            degps[:, h * 512:(h + 1) * 512],
                            ones8[:],
                            msl[:, h * 512:(h + 1) * 512],
                            start=(t == 0), stop=(t == KT - 1))

                # XW1[k, c] = states @ W1 (fp32 PE), packed 8 k-tiles per bank
                with tc.tile_pool(name="xwps", bufs=2, space="PSUM") as xwps:
                    for g in range(KT // 8):
                        ps = xwps.tile([128, 512], dt.float32, tag="xw")
                        for j in range(8):
                            t = g * 8 + j
                            nc.tensor.matmul(
                                ps[:, j * HID:(j + 1) * HID],
                                stT[:, t * 128:(t + 1) * 128],
                                w1[:], start=True, stop=True)
                        nc.scalar.copy(xw1f[:, g * 512:(g + 1) * 512], ps[:])

                # ---- dinv = deg^-1/2 = exp(-0.5 * ln(deg)) (deg is exact) --
                lgdeg = tmp.tile([1, MBLK], dt.float32, tag="sm")
                dinvm = cpool.tile([1, MBLK], dt.float32)
                nc.scalar.activation(lgdeg[:], degps[:], Act.Ln)
                nc.scalar.activation(dinvm[:], lgdeg[:], Act.Exp, scale=-0.5)

            # ---- AllGather dinv across the 8 cores ----
            dinv_src = dram.tile([1, MBLK], dt.float32, name="dinv_src")
            dinv_all = dram.tile([NCORES, MBLK], dt.float32, name="dinv_all",
                                 addr_space="Shared")
            nc.sync.dma_start(dinv_src[:], dinvm[:])
            nc.gpsimd.collective_compute(
                "AllGather", Alu.bypass,
                replica_groups=[list(range(NCORES))],
                ins=[dinv_src[:].opt()], outs=[dinv_all[:].opt()])
            # dinvk[p, t] = dinv[t*128 + p]: contiguous [64,128] load, then
            # PE transpose (a direct strided gather DMA costs ~11us)
            dinvT = tmp.tile([KT, 128], dt.float32, tag="dinvT")
            nc.sync.dma_start(
                dinvT[:],
                dinv_all[:].rearrange("a b -> (a b)").rearrange("(t p) -> t p", p=128))
            with tc.tile_pool(name="trps", bufs=1, space="PSUM") as trps:
                dkps = trps.tile([128, KT], dt.float32, name="dkps")
                nc.tensor.transpose(dkps[:], dinvT[:], id64[:])
                nc.scalar.copy(dinvk[:], dkps[:])

            # dinvbc[c, m] = dinv_m[m] broadcast over 64 partitions (PE trick)
            with tc.tile_pool(name="bcps", bufs=1, space="PSUM") as bcps:
                dbc = bcps.tile([HID, MBLK], dt.float32, name="dbc")
                for h in range(2):
                    nc.tensor.matmul(dbc[:, h * 512:(h + 1) * 512], ones32[:],
                                     dinvm[:, h * 512:(h + 1) * 512],
                                     start=True, stop=True)
                nc.scalar.copy(dinvbc[:], dbc[:])

            # XW1' = dinv[k] * XW1[k, :]  -> fp8
            for t in range(KT):
                nc.vector.tensor_scalar(
                    xw1q[:, t * HID:(t + 1) * HID],
                    xw1f[:, t * HID:(t + 1) * HID],
                    dinvk[:, t:t + 1], None, Alu.mult)
            ph1.release()

            # ================= LAYER 1 SpMM =================
            h1p = cpool.tile([HID, MBLK], dt.bfloat16)
            with tc.tile_pool(name="aggps", bufs=1, space="PSUM") as aggps:
                agg = aggps.tile([HID, MBLK], dt.float32, name="agg1")
                for t in range(KT):
                    for h in range(2):
                        nc.tensor.matmul(
                            agg[:, h * 512:(h + 1) * 512],
                            xw1q[:, t * HID:(t + 1) * HID],
                            mask8[:, t * MBLK + h * 512: t * MBLK + (h + 1) * 512],
                            start=(t == 0), stop=(t == KT - 1))
                # h1 = relu(dinv_m * agg + b1);  h1' = dinv_m * h1 (bf16)
                # halves pipeline the DVE->ACT->DVE chain
                t1 = tmp.tile([HID, MBLK], dt.float32, tag="tt")
                h1T = tmp.tile([HID, MBLK], dt.float32, tag="hh")
                for h in range(2):
                    sl = slice(h * 512, (h + 1) * 512)
                    nc.vector.tensor_tensor(t1[:, sl], agg[:, sl],
                                            dinvbc[:, sl], Alu.mult)
                    nc.scalar.activation(h1T[:, sl], t1[:, sl], Act.Relu,
                                         bias=b1[:])
                    nc.vector.tensor_tensor(h1p[:, sl], h1T[:, sl],
                                            dinvbc[:, sl], Alu.mult)

            # ---- AllGather h1' ----
            h1_src = dram.tile([HID, MBLK], dt.bfloat16, name="h1_src")
            h1_all = dram.tile([NCORES * HID, MBLK], dt.bfloat16, name="h1_all",
                               addr_space="Shared")
            nc.sync.dma_start(h1_src[:, 0:512], h1p[:, 0:512])
            nc.scalar.dma_start(h1_src[:, 512:], h1p[:, 512:])
            nc.gpsimd.collective_compute(
                "AllGather", Alu.bypass,
                replica_groups=[list(range(NCORES))],
                ins=[h1_src[:].opt()], outs=[h1_all[:].opt()])
            for r in range(NCORES):
                nc.sync.dma_start(
                    h1full[:, r * MBLK:(r + 1) * MBLK],
                    h1_all[r * HID:(r + 1) * HID, :])

            # XW2' = h1'^T stationary @ W2 (bf16) -> fp8 (dinv already folded)
            with tc.tile_pool(name="xwps2", bufs=2, space="PSUM") as xwps2:
                for g in range(KT // 8):
                    ps = xwps2.tile([128, 512], dt.float32, tag="xw2")
                    for j in range(8):
                        t = g * 8 + j
                        nc.tensor.matmul(
                            ps[:, j * HID:(j + 1) * HID],
                            h1full[:, t * 128:(t + 1) * 128],
                            w2[:], start=True, stop=True)
                    nc.scalar.copy(xw2q[:, g * 512:(g + 1) * 512], ps[:])

            # ================= LAYER 2 SpMM =================
            h2T = tmp.tile([HID, MBLK], dt.float32, tag="hh")
            with tc.tile_pool(name="aggps2", bufs=1, space="PSUM") as aggps2:
                agg2 = aggps2.tile([HID, MBLK], dt.float32, name="agg2")
                for t in range(KT):
                    for h in range(2):
                        nc.tensor.matmul(
                            agg2[:, h * 512:(h + 1) * 512],
                            xw2q[:, t * HID:(t + 1) * HID],
                            mask8[:, t * MBLK + h * 512: t * MBLK + (h + 1) * 512],
                            start=(t == 0), stop=(t == KT - 1))
                t2 = tmp.tile([HID, MBLK], dt.float32, tag="tt")
                for h in range(2):
                    sl = slice(h * 512, (h + 1) * 512)
                    nc.vector.tensor_tensor(t2[:, sl], agg2[:, sl],
                                            dinvbc[:, sl], Alu.mult)
                    nc.scalar.activation(h2T[:, sl], t2[:, sl], Act.Relu,
                                         bias=b2[:])

            # ================= output head =================
            with tc.tile_pool(name="outps", bufs=1, space="PSUM") as outps:
                ops = outps.tile([1, MBLK], dt.float32, name="ops")
                for h in range(2):
                    nc.tensor.matmul(ops[:, h * 512:(h + 1) * 512], cw[:],
                                     h2T[:, h * 512:(h + 1) * 512],
                                     start=True, stop=True)
                osb = tmp.tile([1, MBLK], dt.float32, tag="sm")
                nc.scalar.activation(osb[:], ops[:], Act.Identity, bias=cb[:])
                nc.sync.dma_start(out_row[:], osb[:])

    nc.compile()
    return nc


def _prep_inputs(states, W1, b1, W2, b2, cw, cb):
    states = np.asarray(states, dtype=np.float32)
    x = np.ascontiguousarray(states[:, 0])
    y = np.ascontiguousarray(states[:, 1])
    bf16 = mybir.dt.np(dt.bfloat16)

    xk = np.ascontiguousarray(x.reshape(KT, 128).T)   # [128, 64]
    yk = np.ascontiguousarray(y.reshape(KT, 128).T)
    stT = np.ascontiguousarray(states.T)              # [4, 8192]
    w1 = np.asarray(W1, dtype=np.float32)
    w2 = np.asarray(W2, dtype=np.float32).astype(bf16)
    b1c = np.asarray(b1, dtype=np.float32).reshape(HID, 1)
    b2c = np.asarray(b2, dtype=np.float32).reshape(HID, 1)
    cwc = np.asarray(cw, dtype=np.float32).reshape(HID, 1)
    cbc = np.asarray(cb, dtype=np.float32).reshape(1, 1)
    id64 = np.eye(HID, dtype=np.float32)

    in_maps = []
    for c in range(NCORES):
        xm = np.ascontiguousarray(
            np.broadcast_to(x[c * MBLK:(c + 1) * MBLK][None, :], (128, MBLK)))
        ym = np.ascontiguousarray(
            np.broadcast_to(y[c * MBLK:(c + 1) * MBLK][None, :], (128, MBLK)))
        in_maps.append({
            "xm_in": xm, "ym_in": ym, "xk_in": xk, "yk_in": yk,
            "stT_in": stT, "w1_in": w1, "w2_in": w2,
            "b1_in": b1c, "b2_in": b2c, "cw_in": cwc, "cb_in": cbc,
            "id64_in": id64,
        })
    return in_maps


def _get_nc():
    if "nc" not in _CACHE:
        _CACHE["nc"] = build_nc()
    return _CACHE["nc"]


def _install_ntff_hook():
    """Recreate the antenv.axon_hooks shim this image is missing."""
    import sys, types
    try:
        from antenv.axon_hooks import get_axon_ntff_profile_hook  # noqa
        return
    except ImportError:
        pass
    try:
        import antenv
        sys.path.insert(0, "/root/.axon_site/trn_agent_boot")
        import trn_boot
        hook = trn_boot._ntff_profile_via_ctypes("/opt/axon/libaxon_pjrt.so")
        mod = types.ModuleType("antenv.axon_hooks")
        mod._hook = hook
        mod.get_axon_ntff_profile_hook = lambda: mod._hook
        mod.set_axon_ntff_profile_hook = lambda h: setattr(mod, "_hook", h)
        sys.modules["antenv.axon_hooks"] = mod
        antenv.axon_hooks = mod
    except Exception as e:  # tracing is best-effort
        print("ntff hook install failed:", e)


def run(trace=False, **inputs):
    if trace:
        _install_ntff_hook()
    nc = _get_nc()
    in_maps = _prep_inputs(**inputs)
    res = bass_utils.run_bass_kernel_spmd(
        nc, in_maps, core_ids=list(range(NCORES)), trace=trace)
    mask = np.concatenate([r["mask_cols"] for r in res.results], axis=1)
    out = np.concatenate(
        [r["out_row"].reshape(MBLK, 1) for r in res.results], axis=0)
    return (out.astype(np.float32), mask.astype(bool)), res


def kernel(**inputs):
    (out, mask), _ = run(trace=False, **inputs)
    return out, mask


# revision 49
# speedup vs baseline: 1.3949x; 1.0089x over previous
"""Trainium2 Bass kernel for nn_CBF (GCN message passing over a radius graph).

8-core SPMD: core c owns agent block m in [c*1024, (c+1)*1024).
Each core builds its [8192 x 1024] column strip of the symmetric adjacency
mask exactly (elementwise fp32, matching jnp rounding), aggregates with the
normalized adjacency via fp8 matmuls on the PE, and all-gathers the small
[N,64] feature matrix between the two GCN layers.

Outputs per core: mask_cols [8192, 1024] u8 (full mask = concat axis=1,
valid because the mask is symmetric... actually because the tiles are
row-indexed by the global k axis directly), out_row [1, 1024] f32.
"""

import numpy as np

import concourse.bass as bass
import concourse.bacc as bacc
import concourse.mybir as mybir
import concourse.tile as tile
from concourse import bass_utils

dt = mybir.dt
Alu = mybir.AluOpType
Act = mybir.ActivationFunctionType

N = 8192
NCORES = 8
MBLK = N // NCORES          # 1024 agents per core
KT = N // 128               # 64 k-tiles of 128 partitions
HID = 64
OBS_R2 = 1.0

_CACHE = {}


def build_nc():
    nc = bacc.Bacc("TRN2", target_bir_lowering=False, debug=False,
                   num_devices=NCORES)

    # ---- kernel I/O (per-core) ----
    xm_in = nc.dram_tensor("xm_in", [128, MBLK], dt.float32, kind="ExternalInput").ap()
    ym_in = nc.dram_tensor("ym_in", [128, MBLK], dt.float32, kind="ExternalInput").ap()
    xk_in = nc.dram_tensor("xk_in", [128, KT], dt.float32, kind="ExternalInput").ap()
    yk_in = nc.dram_tensor("yk_in", [128, KT], dt.float32, kind="ExternalInput").ap()
    stT_in = nc.dram_tensor("stT_in", [4, N], dt.float32, kind="ExternalInput").ap()
    w1_in = nc.dram_tensor("w1_in", [4, HID], dt.float32, kind="ExternalInput").ap()
    w2_in = nc.dram_tensor("w2_in", [HID, HID], dt.bfloat16, kind="ExternalInput").ap()
    b1_in = nc.dram_tensor("b1_in", [HID, 1], dt.float32, kind="ExternalInput").ap()
    b2_in = nc.dram_tensor("b2_in", [HID, 1], dt.float32, kind="ExternalInput").ap()
    cw_in = nc.dram_tensor("cw_in", [HID, 1], dt.float32, kind="ExternalInput").ap()
    cb_in = nc.dram_tensor("cb_in", [1, 1], dt.float32, kind="ExternalInput").ap()
    id64_in = nc.dram_tensor("id64_in", [HID, HID], dt.float32, kind="ExternalInput").ap()

    mask_out = nc.dram_tensor("mask_cols", [N, MBLK], dt.uint8, kind="ExternalOutput").ap()
    out_row = nc.dram_tensor("out_row", [1, MBLK], dt.float32, kind="ExternalOutput").ap()

    with tile.TileContext(nc) as tc:
        with (
            tc.tile_pool(name="const", bufs=1) as cpool,
            tc.tile_pool(name="big", bufs=1) as big,
            tc.tile_pool(name="work", bufs=3) as work,
            tc.tile_pool(name="tmp", bufs=1) as tmp,
            tc.tile_pool(name="dram", bufs=1, space="DRAM") as dram,
        ):
            # ---- load constants; spread DMA *issues* across engine queues
            # (a dma_start issue costs ~600ns serialized on one queue)
            xm = cpool.tile([128, MBLK], dt.float32, name="xm")
            ym = cpool.tile([128, MBLK], dt.float32, name="ym")
            for q, eng in enumerate((nc.sync, nc.scalar, nc.gpsimd, nc.sync)):
                sl = slice(q * MBLK // 4, (q + 1) * MBLK // 4)
                eng.dma_start(xm[:, sl], xm_in[:, sl])
            for q, eng in enumerate((nc.scalar, nc.gpsimd, nc.sync, nc.scalar)):
                sl = slice(q * MBLK // 4, (q + 1) * MBLK // 4)
                eng.dma_start(ym[:, sl], ym_in[:, sl])
            xk = cpool.tile([128, KT], dt.float32, name="xk")
            yk = cpool.tile([128, KT], dt.float32, name="yk")
            nc.gpsimd.dma_start(xk[:], xk_in[:])
            nc.gpsimd.dma_start(yk[:], yk_in[:])
            id64 = cpool.tile([HID, HID], dt.float32, name="id64")
            w1 = cpool.tile([4, HID], dt.float32, name="w1")
            w2 = cpool.tile([HID, HID], dt.bfloat16, name="w2")
            b1 = cpool.tile([HID, 1], dt.float32, name="b1")
            b2 = cpool.tile([HID, 1], dt.float32, name="b2")
            cw = cpool.tile([HID, 1], dt.float32, name="cw")
            cb = cpool.tile([1, 1], dt.float32, name="cb")
            nc.scalar.dma_start(id64[:], id64_in[:])
            nc.scalar.dma_start(w1[:], w1_in[:])
            nc.sync.dma_start(w2[:], w2_in[:])
            nc.sync.dma_start(b1[:], b1_in[:])
            nc.scalar.dma_start(b2[:], b2_in[:])
            nc.sync.dma_start(cw[:], cw_in[:])
            nc.scalar.dma_start(cb[:], cb_in[:])

            ones8 = cpool.tile([128, 1], dt.float8e4)
            nc.gpsimd.memset(ones8[:], 1.0)
            ones32 = cpool.tile([1, HID], dt.float32)
            nc.gpsimd.memset(ones32[:], 1.0)

            # Dummy collective at t~0: prepays the ~11.5us first-collective
            # ncfw warmup so the real dinv AllGather starts promptly.
            warm_in = dram.tile([1, 8], dt.float32, name="warm_in")
            warm_out = dram.tile([NCORES, 8], dt.float32, name="warm_out",
                                 addr_space="Shared")
            wtile = cpool.tile([1, 8], dt.float32)
            nc.gpsimd.memset(wtile[:], 0.0)
            nc.sync.dma_start(warm_in[:], wtile[:])
            # dummy Square: forces the ACT table load at t~0 instead of
            # right before the first real Square
            dsq = cpool.tile([1, 8], dt.float32)
            nc.scalar.activation(dsq[:], wtile[:], Act.Square)
            nc.gpsimd.collective_compute(
                "AllGather", Alu.bypass,
                replica_groups=[list(range(NCORES))],
                ins=[warm_in[:].opt()], outs=[warm_out[:].opt()])

            # ---- persistent big tensors ----
            mask8 = big.tile([128, KT * MBLK], dt.float8e4, name="mask8")   # 64 KB/p
            xw1q = big.tile([128, KT * HID], dt.bfloat16, name="xw1q")      # 8 KB/p
            xw2q = big.tile([128, KT * HID], dt.bfloat16, name="xw2q")      # 8 KB/p
            h1full = big.tile([HID, N], dt.bfloat16, name="h1full")         # 16 KB/p
            dinvk = big.tile([128, KT], dt.float32, name="dinvk")
            dinvbc = big.tile([HID, MBLK], dt.float32, name="dinvbc")

            # ================= PHASE 1: mask + deg (+ XW1 on PE) =========
            ph1 = tc.alloc_tile_pool(name="ph1", bufs=1)
            stT = ph1.tile_from(stT_in)                          # [4, 8192] f32
            xw1f = ph1.tile([128, KT * HID], dt.bfloat16, name="xw1f")  # 8 KB/p
            with tc.tile_pool(name="p1ps", bufs=1, space="PSUM") as p1ps:
                degps = p1ps.tile([1, MBLK], dt.float32, name="degps")

                for t in range(KT):
                    sqx = work.tile([128, MBLK], dt.float32, tag="sqx")
                    sqy = work.tile([128, MBLK], dt.float32, tag="sqy")
                    d2 = work.tile([128, MBLK], dt.float32, tag="d2")
                    # (x_k - x_m)^2 : exact fp32 (scale=-1 mult is exact,
                    # then one rounded add, then Square = one rounded mult)
                    if t % 7 == 3:
                        # ACT/DVE load balance: do the x square on DVE
                        dxt = work.tile([128, MBLK], dt.float32, tag="d2")
                        nc.vector.tensor_scalar(dxt[:], xm[:], xk[:, t:t + 1],
                                                None, Alu.subtract)
                        nc.vector.tensor_tensor(sqx[:], dxt[:], dxt[:], Alu.mult)
                    else:
                        nc.scalar.activation(sqx[:], xm[:], Act.Square,
                                             bias=xk[:, t:t + 1], scale=-1.0)
                    nc.scalar.activation(sqy[:], ym[:], Act.Square,
                                         bias=yk[:, t:t + 1], scale=-1.0)
                    nc.vector.tensor_tensor(d2[:], sqx[:], sqy[:], Alu.add)
                    # mask tile in fp8 (0.0 / 1.0); doubles as the bool
                    # output (raw bytes 0x00/0x38, host decodes with != 0)
                    msl = mask8[:, t * MBLK:(t + 1) * MBLK]
                    nc.vector.tensor_scalar(msl, d2[:], OBS_R2, None, Alu.is_le)
                    if t % 4 == 3:
                        # batched mask write-out: 4 k-tiles per DMA
                        g0 = t - 3
                        src = mask8[:, g0 * MBLK:(t + 1) * MBLK].bitcast(dt.uint8)
                        nc.gpsimd.dma_start(
                            mask_out[g0 * 128:(t + 1) * 128, :].rearrange(
                                "(j p) m -> p j m", p=128),
                            src.rearrange("p (j m) -> p j m", j=4))
                    # deg += ones^T @ mask_tile  (exact integer sums in fp32)
                    for h in range(2):
                        nc.tensor.matmul(
                            degps[:, h * 512:(h + 1) * 512],
                            ones8[:],
                            msl[:, h * 512:(h + 1) * 512],
                            start=(t == 0), stop=(t == KT - 1))

                # XW1[k, c] = states @ W1 (fp32 PE), packed 8 k-tiles per bank
                with tc.tile_pool(name="xwps", bufs=2, space="PSUM") as xwps:
                    for g in range(KT // 8):
                        ps = xwps.tile([128, 512], dt.float32, tag="xw")
                        for j in range(8):
                            t = g * 8 + j
                            nc.tensor.matmul(
                                ps[:, j * HID:(j + 1) * HID],
                                stT[:, t * 128:(t + 1) * 128],
                                w1[:], start=True, stop=True)
                        nc.scalar.copy(xw1f[:, g * 512:(g + 1) * 512], ps[:])

                # ---- dinv = deg^-1/2 = exp(-0.5 * ln(deg)) (deg is exact) --
                lgdeg = tmp.tile([1, MBLK], dt.float32, tag="sm")
                dinvm = cpool.tile([1, MBLK], dt.float32)
                nc.scalar.activation(lgdeg[:], degps[:], Act.Ln)
                nc.scalar.activation(dinvm[:], lgdeg[:], Act.Exp, scale=-0.5)

            # ---- AllGather dinv across the 8 cores ----
            dinv_src = dram.tile([1, MBLK], dt.float32, name="dinv_src")
            dinv_all = dram.tile([NCORES, MBLK], dt.float32, name="dinv_all",
                                 addr_space="Shared")
            nc.sync.dma_start(dinv_src[:], dinvm[:])
            nc.gpsimd.collective_compute(
                "AllGather", Alu.bypass,
                replica_groups=[list(range(NCORES))],
                ins=[dinv_src[:].opt()], outs=[dinv_all[:].opt()])
            # dinvk[p, t] = dinv[t*128 + p]: contiguous [64,128] load, then
            # PE transpose (a direct strided gather DMA costs ~11us)
            dinvT = tmp.tile([KT, 128], dt.float32, tag="dinvT")
            nc.sync.dma_start(
                dinvT[:],
                dinv_all[:].rearrange("a b -> (a b)").rearrange("(t p) -> t p", p=128))
            with tc.tile_pool(name="trps", bufs=1, space="PSUM") as trps:
                dkps = trps.tile([128, KT], dt.float32, name="dkps")
                nc.tensor.transpose(dkps[:], dinvT[:], id64[:])
                nc.scalar.copy(dinvk[:], dkps[:])

            # dinvbc[c, m] = dinv_m[m] broadcast over 64 partitions (PE trick)
            with tc.tile_pool(name="bcps", bufs=1, space="PSUM") as bcps:
                dbc = bcps.tile([HID, MBLK], dt.float32, name="dbc")
                for h in range(2):
                    nc.tensor.matmul(dbc[:, h * 512:(h + 1) * 512], ones32[:],
                                     dinvm[:, h * 512:(h + 1) * 512],
                                     start=True, stop=True)
                nc.scalar.copy(dinvbc[:], dbc[:])

            # XW1' = dinv[k] * XW1[k, :]  -> fp8
            for t in range(KT):
                nc.vector.tensor_scalar(
                    xw1q[:, t * HID:(t + 1) * HID],
                    xw1f[:, t * HID:(t + 1) * HID],
                    dinvk[:, t:t + 1], None, Alu.mult)
            ph1.release()

            # ================= LAYER 1 SpMM =================
            h1p = cpool.tile([HID, MBLK], dt.bfloat16)
            with tc.tile_pool(name="aggps", bufs=1, space="PSUM") as aggps:
                agg = aggps.tile([HID, MBLK], dt.float32, name="agg1")
                for t in range(KT):
                    for h in range(2):
                        nc.tensor.matmul(
                            agg[:, h * 512:(h + 1) * 512],
                            xw1q[:, t * HID:(t + 1) * HID],
                            mask8[:, t * MBLK + h * 512: t * MBLK + (h + 1) * 512],
                            start=(t == 0), stop=(t == KT - 1))
                # h1 = relu(dinv_m * agg + b1);  h1' = dinv_m * h1 (bf16)
                # halves pipeline the DVE->ACT->DVE chain
                t1 = tmp.tile([HID, MBLK], dt.float32, tag="tt")
                h1T = tmp.tile([HID, MBLK], dt.float32, tag="hh")
                for h in range(2):
                    sl = slice(h * 512, (h + 1) * 512)
                    nc.vector.tensor_tensor(t1[:, sl], agg[:, sl],
                                            dinvbc[:, sl], Alu.mult)
                    nc.scalar.activation(h1T[:, sl], t1[:, sl], Act.Relu,
                                         bias=b1[:])
                    nc.vector.tensor_tensor(h1p[:, sl], h1T[:, sl],
                                            dinvbc[:, sl], Alu.mult)

            # ---- AllGather h1' in two m-half chunks so XW2/SpMM2 can start
            # on the first chunk while the second is still gathering.
            # Chunk q carries columns [q*512,(q+1)*512) of every core's h1p
            # = global agent rows {c*1024+q*512 .. +512} = k-tiles with
            # (t % 8) // 4 == q.
            h1_srcs, h1_alls = [], []
            for q in range(2):
                h1_src = dram.tile([HID, 512], dt.bfloat16, name=f"h1_src{q}")
                h1_all = dram.tile([NCORES * HID, 512], dt.bfloat16,
                                   name=f"h1_all{q}", addr_space="Shared")
                h1_srcs.append(h1_src); h1_alls.append(h1_all)
            for q in range(2):
                eng = nc.sync if q == 0 else nc.scalar
                eng.dma_start(h1_srcs[q][:], h1p[:, q * 512:(q + 1) * 512])
                nc.gpsimd.collective_compute(
                    "AllGather", Alu.bypass,
                    replica_groups=[list(range(NCORES))],
                    ins=[h1_srcs[q][:].opt()], outs=[h1_alls[q][:].opt()])
                for r in range(NCORES):
                    nc.sync.dma_start(
                        h1full[:, r * MBLK + q * 512: r * MBLK + (q + 1) * 512],
                        h1_alls[q][r * HID:(r + 1) * HID, :])

            # XW2' = h1'^T stationary @ W2 (bf16); dinv already folded.
            # Process chunk-0 k-tiles first (t%8 in 0..3), then chunk-1.
            tile_order = [t for t in range(KT) if (t % 8) < 4] + \
                         [t for t in range(KT) if (t % 8) >= 4]
            # xw2q is laid out in tile_order position (not t) so the PSUM
            # group drains stay contiguous [128, 512] copies
            with tc.tile_pool(name="xwps2", bufs=2, space="PSUM") as xwps2:
                for g in range(KT // 8):
                    ps = xwps2.tile([128, 512], dt.float32, tag="xw2")
                    for j in range(8):
                        t = tile_order[g * 8 + j]
                        nc.tensor.matmul(
                            ps[:, j * HID:(j + 1) * HID],
                            h1full[:, t * 128:(t + 1) * 128],
                            w2[:], start=True, stop=True)
                    nc.scalar.copy(xw2q[:, g * 512:(g + 1) * 512], ps[:])

            # ================= LAYER 2 SpMM =================
            h2T = tmp.tile([HID, MBLK], dt.float32, tag="hh")
            with tc.tile_pool(name="aggps2", bufs=1, space="PSUM") as aggps2:
                agg2 = aggps2.tile([HID, MBLK], dt.float32, name="agg2")
                for i, t in enumerate(tile_order):
                    for h in range(2):
                        nc.tensor.matmul(
                            agg2[:, h * 512:(h + 1) * 512],
                            xw2q[:, i * HID:(i + 1) * HID],
                            mask8[:, t * MBLK + h * 512: t * MBLK + (h + 1) * 512],
                            start=(i == 0), stop=(i == KT - 1))
                t2 = tmp.tile([HID, MBLK], dt.float32, tag="tt")
                for h in range(2):
                    sl = slice(h * 512, (h + 1) * 512)
                    nc.vector.tensor_tensor(t2[:, sl], agg2[:, sl],
                                            dinvbc[:, sl], Alu.mult)
                    nc.scalar.activation(h2T[:, sl], t2[:, sl], Act.Relu,
                                         bias=b2[:])

            # ================= output head =================
            with tc.tile_pool(name="outps", bufs=1, space="PSUM") as outps:
                ops = outps.tile([1, MBLK], dt.float32, name="ops")
                for h in range(2):
                    nc.tensor.matmul(ops[:, h * 512:(h + 1) * 512], cw[:],
                                     h2T[:, h * 512:(h + 1) * 512],
                                     start=True, stop=True)
                osb = tmp.tile([1, MBLK], dt.float32, tag="sm")
                nc.scalar.activation(osb[:], ops[:], Act.Identity, bias=cb[:])
                nc.sync.dma_start(out_row[:], osb[:])

    nc.compile()
    return nc


def _prep_inputs(states, W1, b1, W2, b2, cw, cb):
    states = np.asarray(states, dtype=np.float32)
    x = np.ascontiguousarray(states[:, 0])
    y = np.ascontiguousarray(states[:, 1])
    bf16 = mybir.dt.np(dt.bfloat16)

    xk = np.ascontiguousarray(x.reshape(KT, 128).T)   # [128, 64]
    yk = np.ascontiguousarray(y.reshape(KT, 128).T)
    stT = np.ascontiguousarray(states.T)              # [4, 8192]
    w1 = np.asarray(W1, dtype=np.float32)
    w2 = np.asarray(W2, dtype=np.float32).astype(bf16)
    b1c = np.asarray(b1, dtype=np.float32).reshape(HID, 1)
    b2c = np.asarray(b2, dtype=np.float32).reshape(HID, 1)
    cwc = np.asarray(cw, dtype=np.float32).reshape(HID, 1)
    cbc = np.asarray(cb, dtype=np.float32).reshape(1, 1)
    id64 = np.eye(HID, dtype=np.float32)

    in_maps = []
    for c in range(NCORES):
        xm = np.ascontiguousarray(
            np.broadcast_to(x[c * MBLK:(c + 1) * MBLK][None, :], (128, MBLK)))
        ym = np.ascontiguousarray(
            np.broadcast_to(y[c * MBLK:(c + 1) * MBLK][None, :], (128, MBLK)))
        in_maps.append({
            "xm_in": xm, "ym_in": ym, "xk_in": xk, "yk_in": yk,
            "stT_in": stT, "w1_in": w1, "w2_in": w2,
            "b1_in": b1c, "b2_in": b2c, "cw_in": cwc, "cb_in": cbc,
            "id64_in": id64,
        })
    return in_maps


def _get_nc():
    if "nc" not in _CACHE:
        _CACHE["nc"] = build_nc()
    return _CACHE["nc"]


def _install_ntff_hook():
    """Recreate the antenv.axon_hooks shim this image is missing."""
    import sys, types
    try:
        from antenv.axon_hooks import get_axon_ntff_profile_hook  # noqa
        return
    except ImportError:
        pass
    try:
        import antenv
        sys.path.insert(0, "/root/.axon_site/trn_agent_boot")
        import trn_boot
        hook = trn_boot._ntff_profile_via_ctypes("/opt/axon/libaxon_pjrt.so")
        mod = types.ModuleType("antenv.axon_hooks")
        mod._hook = hook
        mod.get_axon_ntff_profile_hook = lambda: mod._hook
        mod.set_axon_ntff_profile_hook = lambda h: setattr(mod, "_hook", h)
        sys.modules["antenv.axon_hooks"] = mod
        antenv.axon_hooks = mod
    except Exception as e:  # tracing is best-effort
        print("ntff hook install failed:", e)


def run(trace=False, **inputs):
    if trace:
        _install_ntff_hook()
    nc = _get_nc()
    in_maps = _prep_inputs(**inputs)
    res = bass_utils.run_bass_kernel_spmd(
        nc, in_maps, core_ids=list(range(NCORES)), trace=trace)
    mask = np.concatenate([r["mask_cols"] for r in res.results], axis=1)
    out = np.concatenate(
        [r["out_row"].reshape(MBLK, 1) for r in res.results], axis=0)
    return (out.astype(np.float32), mask.astype(bool)), res


def kernel(**inputs):
    (out, mask), _ = run(trace=False, **inputs)
    return out, mask
